# revision 11
# baseline (speedup 1.0000x reference)
"""CLAM (gated-attention MIL) kernel for 8x Trainium2 NeuronCores.

Strategy: shard the N=50000 patch axis across the 8 cores (6250 rows each).
Per core, a single Bass/Tile kernel streams each bag's shard in row-blocks:

    x^T  = relu(Wfc^T @ h^T + bfc)            (PE fp32r + ACT)
    a^T/b^T = tanh/sigmoid(Wab^T @ x^T + b)   (PE fp32r + ACT)
    A    = (a*b)^T contracted with Wc          (DVE mul + PE)
    e    = exp(A + bc), s += sum(e)            (ACT with accum)
    u   += x^T @ e  (softmax numerator)        (PE broadcast + DVE fused
                                                mul-reduce along rows)

h is transposed on the host during sharding so the contraction dim (L) lands
on the SBUF partition axis; all matmuls use float32r (TF32-like, full rate,
~1e-4 rel err). The softmax normalization, attention pooling, top-k and
instance loss are finished on the host from the per-core partials (exact
math; top-k x-rows are recomputed from h4 in fp32 on host).
"""

import numpy as np

N, L, H, D, NCLS, K = 50000, 1024, 512, 256, 2, 8
NCORES = 8
NR = N // NCORES                      # 6250 rows per core
BLOCKS = [512] * 11 + [310, 308]      # row-block widths (>=256 for fp32r full rate, even for fp32r ISA)
NBLK = len(BLOCKS)
KC = L // 128                         # 8 contraction chunks for fc
HC = H // 128                         # 4 output chunks of x^T
D2 = 2 * D                            # Wa|Wb concat
DC2 = D2 // 128                       # 4 output chunks of (a|b)^T
DCC = D // 128                        # 2 chunks of the D contraction

_BUILT = {}


def _build_kernel(nr=NR, blocks=None, stage=6):
    key = (nr, tuple(blocks) if blocks else None, stage)
    if key in _BUILT:
        return _BUILT[key], None
    import concourse.bacc as bacc
    import concourse.mybir as mybir
    from concourse.tile import TileContext

    F32 = mybir.dt.float32
    F32R = mybir.dt.float32r
    AF = mybir.ActivationFunctionType
    ALU = mybir.AluOpType

    if blocks is None:
        blocks = BLOCKS
    nblk = len(blocks)
    assert sum(blocks) == nr

    nc = bacc.Bacc("TRN2", target_bir_lowering=False, debug=False)

    hts = [nc.dram_tensor(f"h{i}t", [L, nr], F32R, kind="ExternalInput") for i in range(4)]
    wfc = nc.dram_tensor("wfc", [128, KC * H], F32R, kind="ExternalInput")
    bfc = nc.dram_tensor("bfc", [128, HC], F32, kind="ExternalInput")
    wab = nc.dram_tensor("wab", [128, HC * D2], F32R, kind="ExternalInput")
    bab = nc.dram_tensor("bab", [128, DC2], F32, kind="ExternalInput")
    wc = nc.dram_tensor("wc", [128, 128 * DCC], F32R, kind="ExternalInput")
    bc = nc.dram_tensor("bc", [128, 1], F32, kind="ExternalInput")

    a4_d = nc.dram_tensor("a4", [1, nr], F32, kind="ExternalOutput")
    s_d = nc.dram_tensor("sparts", [128, 4 * nblk], F32, kind="ExternalOutput")
    u_d = nc.dram_tensor("uparts", [128, 4 * HC * nblk], F32, kind="ExternalOutput")

    with TileContext(nc) as tc:
        with (
            tc.tile_pool(name="const", bufs=1) as cpool,
            tc.tile_pool(name="h", bufs=3) as hpool,
            tc.tile_pool(name="x", bufs=2) as xpool,
            tc.tile_pool(name="ab", bufs=2) as abpool,
            tc.tile_pool(name="pr", bufs=2) as prpool,
            tc.tile_pool(name="e", bufs=2) as epool,
            tc.tile_pool(name="scr", bufs=2) as scrpool,
            tc.tile_pool(name="acc", bufs=1) as accpool,
            tc.tile_pool(name="psx", bufs=3, space="PSUM") as psx_pool,
            tc.tile_pool(name="psab", bufs=3, space="PSUM") as psab_pool,
            tc.tile_pool(name="psA", bufs=2, space="PSUM") as psA_pool,
        ):
            wfc_sb = cpool.tile([128, KC * H], F32R)
            for kc in range(KC):
                for hc in range(HC):
                    o = kc * H + hc * 128
                    nc.sync.dma_start(out=wfc_sb[:, o:o + 128], in_=wfc.ap()[:, o:o + 128])
            bfc_sb = cpool.tile([128, HC], F32)
            nc.sync.dma_start(out=bfc_sb, in_=bfc.ap())
            wab_sb = cpool.tile([128, HC * D2], F32R)
            for hc in range(HC):
                for dc in range(DC2):
                    o = hc * D2 + dc * 128
                    nc.sync.dma_start(out=wab_sb[:, o:o + 128], in_=wab.ap()[:, o:o + 128])
            bab_sb = cpool.tile([128, DC2], F32)
            nc.sync.dma_start(out=bab_sb, in_=bab.ap())
            wc_sb = cpool.tile([128, 128 * DCC], F32R)
            nc.sync.dma_start(out=wc_sb, in_=wc.ap())
            bc_sb = cpool.tile([128, 1], F32)
            nc.sync.dma_start(out=bc_sb, in_=bc.ap())
            bc_bcast = bc_sb[:, 0:1]

            a4_sb = accpool.tile([1, nr], F32)
            s_all = accpool.tile([128, 4 * nblk], F32)
            u_all = accpool.tile([128, 4 * HC * nblk], F32)
            nc.vector.memset(a4_sb, 0.0)
            nc.vector.memset(s_all, 0.0)
            nc.vector.memset(u_all, 0.0)

            for bag in range(4):
                c0 = 0
                for b, RB in enumerate(blocks):
                    hblk = hpool.tile([128, KC * RB], F32R, tag="h", name=f"h_{bag}_{b}")
                    for kc in range(KC):
                        nc.sync.dma_start(
                            out=hblk[:, kc * RB:(kc + 1) * RB],
                            in_=hts[bag].ap()[kc * 128:(kc + 1) * 128, c0:c0 + RB],
                        )
                    xblk = xpool.tile([128, HC * RB], F32R, tag="x", name=f"x_{bag}_{b}")
                    for hc in range(HC):
                        psx = psx_pool.tile([128, RB], F32, tag="psx", name=f"psx_{bag}_{b}_{hc}")
                        for kc in range(KC):
                            nc.tensor.matmul(
                                out=psx,
                                lhsT=wfc_sb[:, kc * H + hc * 128:kc * H + (hc + 1) * 128],
                                rhs=hblk[:, kc * RB:(kc + 1) * RB],
                                start=(kc == 0),
                                stop=(kc == KC - 1),
                            )
                        nc.scalar.activation(
                            xblk[:, hc * RB:(hc + 1) * RB], psx, AF.Relu,
                            bias=bfc_sb[:, hc:hc + 1],
                        )
                    if stage < 2:
                        c0 += RB
                        continue
                    abblk = abpool.tile([128, DC2 * RB], F32, tag="ab", name=f"ab_{bag}_{b}")
                    for dc in range(DC2):
                        psab = psab_pool.tile([128, RB], F32, tag="psab", name=f"psab_{bag}_{b}_{dc}")
                        for hc in range(HC):
                            nc.tensor.matmul(
                                out=psab,
                                lhsT=wab_sb[:, hc * D2 + dc * 128:hc * D2 + (dc + 1) * 128],
                                rhs=xblk[:, hc * RB:(hc + 1) * RB],
                                start=(hc == 0),
                                stop=(hc == HC - 1),
                            )
                        nc.scalar.activation(
                            abblk[:, dc * RB:(dc + 1) * RB], psab,
                            AF.Tanh if dc < DCC else AF.Sigmoid,
                            bias=bab_sb[:, dc:dc + 1],
                        )
                    abprod = prpool.tile([128, DCC * RB], F32R, tag="pr", name=f"pr_{bag}_{b}")
                    for dc in range(DCC):
                        nc.vector.tensor_mul(
                            out=abprod[:, dc * RB:(dc + 1) * RB],
                            in0=abblk[:, dc * RB:(dc + 1) * RB],
                            in1=abblk[:, (DCC + dc) * RB:(DCC + dc + 1) * RB],
                        )
                    if stage < 3:
                        c0 += RB
                        continue
                    psA = psA_pool.tile([128, RB], F32, tag="psA", name=f"psA_{bag}_{b}")
                    for dc in range(DCC):
                        nc.tensor.matmul(
                            out=psA,
                            lhsT=wc_sb[:, 128 * dc:128 * (dc + 1)],
                            rhs=abprod[:, dc * RB:(dc + 1) * RB],
                            start=(dc == 0),
                            stop=(dc == DCC - 1),
                        )
                    if stage < 4:
                        c0 += RB
                        continue
                    e_sb = epool.tile([128, RB], F32, tag="e", name=f"e_{bag}_{b}")
                    nc.scalar.activation(
                        e_sb, psA, AF.Exp, bias=bc_bcast,
                        accum_out=s_all[:, bag * nblk + b:bag * nblk + b + 1],
                    )
                    if bag == 3:
                        nc.scalar.activation(
                            a4_sb[:, c0:c0 + RB], psA[0:1, :], AF.Identity, bias=bc_sb[0:1, 0:1],
                        )
                    if stage < 6:
                        c0 += RB
                        continue
                    for hc in range(HC):
                        scr = scrpool.tile([128, RB], F32, tag="scr", name=f"scr_{bag}_{b}_{hc}")
                        col = (bag * HC + hc) * nblk + b
                        # NB: tensor_tensor_reduce hard-faults the HW here;
                        # use separate mul + free-axis reduce instead.
                        nc.vector.tensor_mul(
                            out=scr,
                            in0=xblk[:, hc * RB:(hc + 1) * RB].bitcast(F32),
                            in1=e_sb,
                        )
                        nc.vector.tensor_reduce(
                            out=u_all[:, col:col + 1],
                            in_=scr,
                            axis=mybir.AxisListType.X,
                            op=ALU.add,
                        )
                    c0 += RB

            nc.sync.dma_start(out=a4_d.ap(), in_=a4_sb)
            nc.sync.dma_start(out=s_d.ap(), in_=s_all)
            nc.sync.dma_start(out=u_d.ap(), in_=u_all)

    nc.compile()
    _BUILT[key] = nc
    return nc, None


def _prep_weights(Wfc, bfc, Wa, ba, Wb, bb, Wc, bc):
    f32 = np.float32
    wfc_t = np.ascontiguousarray(
        Wfc.astype(f32).reshape(KC, 128, H).transpose(1, 0, 2).reshape(128, KC * H))
    bfc_t = np.ascontiguousarray(bfc.astype(f32).reshape(HC, 128).T)
    Wab = np.concatenate([Wa.astype(f32), Wb.astype(f32)], axis=1)      # [H, 2D]
    wab_t = np.ascontiguousarray(
        Wab.reshape(HC, 128, D2).transpose(1, 0, 2).reshape(128, HC * D2))
    bab = np.concatenate([ba.astype(f32), bb.astype(f32)])              # [2D]
    bab_t = np.ascontiguousarray(bab.reshape(DC2, 128).T)
    wc_t = np.zeros((128, 128 * DCC), f32)
    for dc in range(DCC):
        wc_t[:, dc * 128:(dc + 1) * 128] = Wc.astype(f32)[dc * 128:(dc + 1) * 128, 0:1]
    bc_t = np.broadcast_to(bc.astype(f32).reshape(1, 1), (128, 1)).copy()
    return wfc_t, bfc_t, wab_t, bab_t, wc_t, bc_t


def kernel(h1, h2, h3, h4, Wfc, bfc, Wa, ba, Wb, bb, Wc, bc,
           Wattn, battn, Wcls, bcls, Wic, bic):
    from concourse.bass_utils import run_bass_kernel_spmd

    nc, _ = _build_kernel()
    f32 = np.float32
    wfc_t, bfc_t, wab_t, bab_t, wc_t, bc_t = _prep_weights(
        Wfc, bfc, Wa, ba, Wb, bb, Wc, bc)

    hs = [np.asarray(h, f32) for h in (h1, h2, h3, h4)]
    in_maps = []
    for c in range(NCORES):
        m = {
            "wfc": wfc_t, "bfc": bfc_t, "wab": wab_t, "bab": bab_t,
            "wc": wc_t, "bc": bc_t,
        }
        for i in range(4):
            m[f"h{i}t"] = np.ascontiguousarray(hs[i][c * NR:(c + 1) * NR, :].T)
        in_maps.append(m)

    res = run_bass_kernel_spmd(nc, in_maps, core_ids=list(range(NCORES)))

    # ---- host-side finish (float64 for the cheap exact parts) ----
    A4 = np.concatenate([res.results[c]["a4"][0] for c in range(NCORES)])  # [N]
    s_bag = np.zeros(4, np.float64)
    u_bag = np.zeros((4, H), np.float64)
    for c in range(NCORES):
        sp = res.results[c]["sparts"][0].astype(np.float64)        # row 0 of [128, 4*NBLK]
        up = res.results[c]["uparts"].astype(np.float64)           # [128, 4*HC*NBLK]
        for bag in range(4):
            s_bag[bag] += sp[bag * NBLK:(bag + 1) * NBLK].sum()
            for hc in range(HC):
                cols = up[:, (bag * HC + hc) * NBLK:(bag * HC + hc + 1) * NBLK]
                u_bag[bag, hc * 128:(hc + 1) * 128] += cols.sum(axis=1)

    Ms = (u_bag / s_bag[:, None]).astype(f32)                      # [4, H]

    combined = Ms.reshape(1, 4 * H)
    logits_attn = combined @ Wattn.astype(f32) + battn.astype(f32)
    w = _softmax(logits_attn)                                      # [1, 4]
    M = (w @ Ms).astype(f32)                                       # [1, H]
    logits = M @ Wcls.astype(f32) + bcls.astype(f32)               # [1, NCLS]
    Y_prob = _softmax(logits).astype(f32)
    Y_hat = np.argmax(logits, axis=1).astype(np.int32)
    logits = logits.astype(f32)

    # ---- instance path (bag 4): top-k from device scores, x recomputed ----
    order = np.argsort(-A4, kind="stable")
    top_p = order[:K]
    order_lo = np.argsort(A4, kind="stable")
    top_n = order_lo[:K]
    ids = np.concatenate([top_p, top_n])
    x_sel = np.maximum(hs[3][ids] @ Wfc.astype(f32) + bfc.astype(f32), 0.0)
    inst_logits = x_sel @ Wic.astype(f32) + bic.astype(f32)        # [2K, 2]
    mx = inst_logits.max(axis=1, keepdims=True)
    lse = mx + np.log(np.exp(inst_logits - mx).sum(axis=1, keepdims=True))
    logp = inst_logits - lse
    targets = np.concatenate([np.ones(K, np.int32), np.zeros(K, np.int32)])
    inst_loss = np.float32(-logp[np.arange(2 * K), targets].mean())

    A_raw = A4.reshape(1, N).astype(f32)
    return logits, Y_prob, Y_hat, A_raw, inst_loss


def _softmax(x):
    x = np.asarray(x, np.float64)
    m = x.max(axis=1, keepdims=True)
    e = np.exp(x - m)
    return e / e.sum(axis=1, keepdims=True)


# revision 12
# speedup vs baseline: 1.0441x; 1.0441x over previous
"""CLAM (gated-attention MIL) kernel for 8x Trainium2 NeuronCores.

Strategy: shard the N=50000 patch axis across the 8 cores (6250 rows each).
Per core, a single Bass/Tile kernel streams each bag's shard in row-blocks:

    x^T  = relu(Wfc^T @ h^T + bfc)            (PE fp32r + ACT)
    a^T/b^T = tanh/sigmoid(Wab^T @ x^T + b)   (PE fp32r + ACT)
    A    = (a*b)^T contracted with Wc          (DVE mul + PE)
    e    = exp(A + bc), s += sum(e)            (ACT with accum)
    u   += x^T @ e  (softmax numerator)        (PE broadcast + DVE fused
                                                mul-reduce along rows)

h is transposed on the host during sharding so the contraction dim (L) lands
on the SBUF partition axis; all matmuls use float32r (TF32-like, full rate,
~1e-4 rel err). The softmax normalization, attention pooling, top-k and
instance loss are finished on the host from the per-core partials (exact
math; top-k x-rows are recomputed from h4 in fp32 on host).
"""

import numpy as np

N, L, H, D, NCLS, K = 50000, 1024, 512, 256, 2, 8
NCORES = 8
NR = N // NCORES                      # 6250 rows per core
BLOCKS = [512] * 11 + [310, 308]      # row-block widths (>=256 for fp32r full rate, even for fp32r ISA)
NBLK = len(BLOCKS)
KC = L // 128                         # 8 contraction chunks for fc
HC = H // 128                         # 4 output chunks of x^T
D2 = 2 * D                            # Wa|Wb concat
DC2 = D2 // 128                       # 4 output chunks of (a|b)^T
DCC = D // 128                        # 2 chunks of the D contraction

_BUILT = {}


def _build_kernel(nr=NR, blocks=None, stage=6):
    key = (nr, tuple(blocks) if blocks else None, stage)
    if key in _BUILT:
        return _BUILT[key], None
    import concourse.bacc as bacc
    import concourse.mybir as mybir
    from concourse.tile import TileContext

    F32 = mybir.dt.float32
    F32R = mybir.dt.float32r
    AF = mybir.ActivationFunctionType
    ALU = mybir.AluOpType

    if blocks is None:
        blocks = BLOCKS
    nblk = len(blocks)
    assert sum(blocks) == nr

    nc = bacc.Bacc("TRN2", target_bir_lowering=False, debug=False)

    hts = [nc.dram_tensor(f"h{i}t", [L, nr], F32R, kind="ExternalInput") for i in range(4)]
    wfc = nc.dram_tensor("wfc", [128, KC * H], F32R, kind="ExternalInput")
    bfc = nc.dram_tensor("bfc", [128, HC], F32, kind="ExternalInput")
    wab = nc.dram_tensor("wab", [128, HC * D2], F32R, kind="ExternalInput")
    bab = nc.dram_tensor("bab", [128, DC2], F32, kind="ExternalInput")
    wc = nc.dram_tensor("wc", [128, 128 * DCC], F32R, kind="ExternalInput")
    bc = nc.dram_tensor("bc", [128, 1], F32, kind="ExternalInput")

    a4_d = nc.dram_tensor("a4", [1, nr], F32, kind="ExternalOutput")
    s_d = nc.dram_tensor("sparts", [128, 4 * nblk], F32, kind="ExternalOutput")
    u_d = nc.dram_tensor("uparts", [128, 4 * HC * nblk], F32, kind="ExternalOutput")

    with TileContext(nc) as tc:
        with (
            tc.tile_pool(name="const", bufs=1) as cpool,
            tc.tile_pool(name="h", bufs=3) as hpool,
            tc.tile_pool(name="x", bufs=2) as xpool,
            tc.tile_pool(name="ab", bufs=2) as abpool,
            tc.tile_pool(name="pr", bufs=2) as prpool,
            tc.tile_pool(name="e", bufs=2) as epool,
            tc.tile_pool(name="scr", bufs=2) as scrpool,
            tc.tile_pool(name="acc", bufs=1) as accpool,
            tc.tile_pool(name="psx", bufs=3, space="PSUM") as psx_pool,
            tc.tile_pool(name="psab", bufs=3, space="PSUM") as psab_pool,
            tc.tile_pool(name="psA", bufs=2, space="PSUM") as psA_pool,
        ):
            wfc_sb = cpool.tile([128, KC * H], F32R)
            nc.sync.dma_start(out=wfc_sb, in_=wfc.ap())
            bfc_sb = cpool.tile([128, HC], F32)
            nc.sync.dma_start(out=bfc_sb, in_=bfc.ap())
            wab_sb = cpool.tile([128, HC * D2], F32R)
            nc.sync.dma_start(out=wab_sb, in_=wab.ap())
            bab_sb = cpool.tile([128, DC2], F32)
            nc.sync.dma_start(out=bab_sb, in_=bab.ap())
            wc_sb = cpool.tile([128, 128 * DCC], F32R)
            nc.sync.dma_start(out=wc_sb, in_=wc.ap())
            bc_sb = cpool.tile([128, 1], F32)
            nc.sync.dma_start(out=bc_sb, in_=bc.ap())
            bc_bcast = bc_sb[:, 0:1]

            a4_sb = accpool.tile([1, nr], F32)
            s_all = accpool.tile([128, 4 * nblk], F32)
            u_all = accpool.tile([128, 4 * HC * nblk], F32)
            nc.vector.memset(a4_sb, 0.0)
            nc.vector.memset(s_all, 0.0)
            nc.vector.memset(u_all, 0.0)

            for bag in range(4):
                c0 = 0
                for b, RB in enumerate(blocks):
                    hblk = hpool.tile([128, KC * RB], F32R, tag="h", name=f"h_{bag}_{b}")
                    for kc in range(KC):
                        nc.sync.dma_start(
                            out=hblk[:, kc * RB:(kc + 1) * RB],
                            in_=hts[bag].ap()[kc * 128:(kc + 1) * 128, c0:c0 + RB],
                        )
                    xblk = xpool.tile([128, HC * RB], F32R, tag="x", name=f"x_{bag}_{b}")
                    for hc in range(HC):
                        psx = psx_pool.tile([128, RB], F32, tag="psx", name=f"psx_{bag}_{b}_{hc}")
                        for kc in range(KC):
                            nc.tensor.matmul(
                                out=psx,
                                lhsT=wfc_sb[:, kc * H + hc * 128:kc * H + (hc + 1) * 128],
                                rhs=hblk[:, kc * RB:(kc + 1) * RB],
                                start=(kc == 0),
                                stop=(kc == KC - 1),
                            )
                        nc.scalar.activation(
                            xblk[:, hc * RB:(hc + 1) * RB], psx, AF.Relu,
                            bias=bfc_sb[:, hc:hc + 1],
                        )
                    if stage < 2:
                        c0 += RB
                        continue
                    abblk = abpool.tile([128, DC2 * RB], F32, tag="ab", name=f"ab_{bag}_{b}")
                    for dc in range(DC2):
                        psab = psab_pool.tile([128, RB], F32, tag="psab", name=f"psab_{bag}_{b}_{dc}")
                        for hc in range(HC):
                            nc.tensor.matmul(
                                out=psab,
                                lhsT=wab_sb[:, hc * D2 + dc * 128:hc * D2 + (dc + 1) * 128],
                                rhs=xblk[:, hc * RB:(hc + 1) * RB],
                                start=(hc == 0),
                                stop=(hc == HC - 1),
                            )
                        nc.scalar.activation(
                            abblk[:, dc * RB:(dc + 1) * RB], psab,
                            AF.Tanh if dc < DCC else AF.Sigmoid,
                            bias=bab_sb[:, dc:dc + 1],
                        )
                    abprod = prpool.tile([128, DCC * RB], F32R, tag="pr", name=f"pr_{bag}_{b}")
                    for dc in range(DCC):
                        nc.vector.tensor_mul(
                            out=abprod[:, dc * RB:(dc + 1) * RB],
                            in0=abblk[:, dc * RB:(dc + 1) * RB],
                            in1=abblk[:, (DCC + dc) * RB:(DCC + dc + 1) * RB],
                        )
                    if stage < 3:
                        c0 += RB
                        continue
                    psA = psA_pool.tile([128, RB], F32, tag="psA", name=f"psA_{bag}_{b}")
                    for dc in range(DCC):
                        nc.tensor.matmul(
                            out=psA,
                            lhsT=wc_sb[:, 128 * dc:128 * (dc + 1)],
                            rhs=abprod[:, dc * RB:(dc + 1) * RB],
                            start=(dc == 0),
                            stop=(dc == DCC - 1),
                        )
                    if stage < 4:
                        c0 += RB
                        continue
                    e_sb = epool.tile([128, RB], F32, tag="e", name=f"e_{bag}_{b}")
                    nc.scalar.activation(
                        e_sb, psA, AF.Exp, bias=bc_bcast,
                        accum_out=s_all[:, bag * nblk + b:bag * nblk + b + 1],
                    )
                    if bag == 3:
                        nc.scalar.activation(
                            a4_sb[:, c0:c0 + RB], psA[0:1, :], AF.Identity, bias=bc_sb[0:1, 0:1],
                        )
                    if stage < 6:
                        c0 += RB
                        continue
                    for hc in range(HC):
                        scr = scrpool.tile([128, RB], F32, tag="scr", name=f"scr_{bag}_{b}_{hc}")
                        col = (bag * HC + hc) * nblk + b
                        # NB: tensor_tensor_reduce hard-faults the HW here;
                        # use separate mul + free-axis reduce instead.
                        nc.vector.tensor_mul(
                            out=scr,
                            in0=xblk[:, hc * RB:(hc + 1) * RB].bitcast(F32),
                            in1=e_sb,
                        )
                        nc.vector.tensor_reduce(
                            out=u_all[:, col:col + 1],
                            in_=scr,
                            axis=mybir.AxisListType.X,
                            op=ALU.add,
                        )
                    c0 += RB

            nc.sync.dma_start(out=a4_d.ap(), in_=a4_sb)
            nc.sync.dma_start(out=s_d.ap(), in_=s_all)
            nc.sync.dma_start(out=u_d.ap(), in_=u_all)

    nc.compile()
    _BUILT[key] = nc
    return nc, None


def _prep_weights(Wfc, bfc, Wa, ba, Wb, bb, Wc, bc):
    f32 = np.float32
    wfc_t = np.ascontiguousarray(
        Wfc.astype(f32).reshape(KC, 128, H).transpose(1, 0, 2).reshape(128, KC * H))
    bfc_t = np.ascontiguousarray(bfc.astype(f32).reshape(HC, 128).T)
    Wab = np.concatenate([Wa.astype(f32), Wb.astype(f32)], axis=1)      # [H, 2D]
    wab_t = np.ascontiguousarray(
        Wab.reshape(HC, 128, D2).transpose(1, 0, 2).reshape(128, HC * D2))
    bab = np.concatenate([ba.astype(f32), bb.astype(f32)])              # [2D]
    bab_t = np.ascontiguousarray(bab.reshape(DC2, 128).T)
    wc_t = np.zeros((128, 128 * DCC), f32)
    for dc in range(DCC):
        wc_t[:, dc * 128:(dc + 1) * 128] = Wc.astype(f32)[dc * 128:(dc + 1) * 128, 0:1]
    bc_t = np.broadcast_to(bc.astype(f32).reshape(1, 1), (128, 1)).copy()
    return wfc_t, bfc_t, wab_t, bab_t, wc_t, bc_t


def kernel(h1, h2, h3, h4, Wfc, bfc, Wa, ba, Wb, bb, Wc, bc,
           Wattn, battn, Wcls, bcls, Wic, bic):
    from concourse.bass_utils import run_bass_kernel_spmd

    nc, _ = _build_kernel()
    f32 = np.float32
    wfc_t, bfc_t, wab_t, bab_t, wc_t, bc_t = _prep_weights(
        Wfc, bfc, Wa, ba, Wb, bb, Wc, bc)

    hs = [np.asarray(h, f32) for h in (h1, h2, h3, h4)]
    in_maps = []
    for c in range(NCORES):
        m = {
            "wfc": wfc_t, "bfc": bfc_t, "wab": wab_t, "bab": bab_t,
            "wc": wc_t, "bc": bc_t,
        }
        for i in range(4):
            m[f"h{i}t"] = np.ascontiguousarray(hs[i][c * NR:(c + 1) * NR, :].T)
        in_maps.append(m)

    res = run_bass_kernel_spmd(nc, in_maps, core_ids=list(range(NCORES)))

    # ---- host-side finish (float64 for the cheap exact parts) ----
    A4 = np.concatenate([res.results[c]["a4"][0] for c in range(NCORES)])  # [N]
    s_bag = np.zeros(4, np.float64)
    u_bag = np.zeros((4, H), np.float64)
    for c in range(NCORES):
        sp = res.results[c]["sparts"][0].astype(np.float64)        # row 0 of [128, 4*NBLK]
        up = res.results[c]["uparts"].astype(np.float64)           # [128, 4*HC*NBLK]
        for bag in range(4):
            s_bag[bag] += sp[bag * NBLK:(bag + 1) * NBLK].sum()
            for hc in range(HC):
                cols = up[:, (bag * HC + hc) * NBLK:(bag * HC + hc + 1) * NBLK]
                u_bag[bag, hc * 128:(hc + 1) * 128] += cols.sum(axis=1)

    Ms = (u_bag / s_bag[:, None]).astype(f32)                      # [4, H]

    combined = Ms.reshape(1, 4 * H)
    logits_attn = combined @ Wattn.astype(f32) + battn.astype(f32)
    w = _softmax(logits_attn)                                      # [1, 4]
    M = (w @ Ms).astype(f32)                                       # [1, H]
    logits = M @ Wcls.astype(f32) + bcls.astype(f32)               # [1, NCLS]
    Y_prob = _softmax(logits).astype(f32)
    Y_hat = np.argmax(logits, axis=1).astype(np.int32)
    logits = logits.astype(f32)

    # ---- instance path (bag 4): top-k from device scores, x recomputed ----
    order = np.argsort(-A4, kind="stable")
    top_p = order[:K]
    order_lo = np.argsort(A4, kind="stable")
    top_n = order_lo[:K]
    ids = np.concatenate([top_p, top_n])
    x_sel = np.maximum(hs[3][ids] @ Wfc.astype(f32) + bfc.astype(f32), 0.0)
    inst_logits = x_sel @ Wic.astype(f32) + bic.astype(f32)        # [2K, 2]
    mx = inst_logits.max(axis=1, keepdims=True)
    lse = mx + np.log(np.exp(inst_logits - mx).sum(axis=1, keepdims=True))
    logp = inst_logits - lse
    targets = np.concatenate([np.ones(K, np.int32), np.zeros(K, np.int32)])
    inst_loss = np.float32(-logp[np.arange(2 * K), targets].mean())

    A_raw = A4.reshape(1, N).astype(f32)
    return logits, Y_prob, Y_hat, A_raw, inst_loss


def _softmax(x):
    x = np.asarray(x, np.float64)
    m = x.max(axis=1, keepdims=True)
    e = np.exp(x - m)
    return e / e.sum(axis=1, keepdims=True)
